# revision 1
# baseline (speedup 1.0000x reference)
"""Trainium2 Bass kernel for a 3-layer GCN bottleneck block (50k nodes, 800k edges).

Strategy (8 NeuronCores, dst-node sharding):
- Host: relabel nodes into 8 cores x TILES tiles x 128 slots, balancing per-tile
  in-degree. Edges sorted by (dst tile, src half, src id); each (tile, half)
  group padded to a uniform chunk count so one SPMD program serves all cores.
  Self-loops become plain edges.
- All three convs aggregate in 64-dim space (conv3 rewritten as (A~ @ a2) @ W3).
- Per layer: compute local h-shard = a @ W with dinv[src] folded in, AllGather
  the [NPAD, 64] fp32 table, dma_gather 256B rows per edge (src-sorted for HBM
  locality), segment-sum via one-hot matmul: aggT[64f,128d] += M16.T @ S where
  S = is_equal(iota, dstloc) built in one DVE pass per batch (pads get dstloc=-1
  so their S column is zero). dinv[dst] applied during PSUM eviction.
- BatchNorm: per-core feature-major partial sums, tiny AllReduce, ACT-fused
  scale/bias/relu. Final layer: W3 matmul to 256-dim, BN3, residual, relu,
  output transposed; host untransposes/unpermutes.
"""

import os
import numpy as np

DIN = 256
DOUT = 64
EPS = 1e-5
NCORE = 8

# default (real-problem) geometry; test_sim.py overrides via configure()
_CFG = {}


def configure(N, E, NLOC, BTILES):
    TILES = (NLOC + 127) // 128
    NP = TILES * 128
    _CFG.update(
        N=N, E=E, NLOC=NLOC, TILES=TILES, NP=NP,
        NPAD=NCORE * NP, HALF=(NCORE * NP) // 2,
        LASTV=NLOC - 128 * (TILES - 1),       # valid rows in last tile
        BTILES=BTILES,
    )


configure(N=50000, E=800000, NLOC=6250, BTILES=3)

LAST_EXEC_NS = None    # set by kernel() when GCN_TRACE=1


# ----------------------------------------------------------------------------
# host-side graph prep
# ----------------------------------------------------------------------------

def _prep_graph(ei):
    import heapq
    N, TILES, NP = _CFG["N"], _CFG["TILES"], _CFG["NP"]
    LASTV = _CFG["LASTV"]
    src, dst = ei[0].astype(np.int64), ei[1].astype(np.int64)
    deg = np.bincount(dst, minlength=N).astype(np.float32) + 1.0
    dinv = (1.0 / np.sqrt(deg)).astype(np.float32)
    indeg = np.bincount(dst, minlength=N)

    # Balance BOTH lo and hi in-edge loads per (core,tile) slot: the padded
    # chunk counts K_lo/K_hi are set by the max over slots, so minimize the
    # larger of the two running loads. lo/hi of an edge depends on the SOURCE
    # node's final core, unknown during assignment — approximate with a
    # first-pass assignment by total degree, then rebalance on realized lo/hi.
    nslot = NCORE * TILES
    cap = np.full(nslot, 128, np.int64)
    cap[TILES - 1 :: TILES] = LASTV

    def greedy(key_lo, key_hi):
        order = np.argsort(-(key_lo + key_hi), kind="stable")
        load_lo = np.zeros(nslot, np.float64)
        load_hi = np.zeros(nslot, np.float64)
        fill = np.zeros(nslot, np.int64)
        slot_of = np.empty(N, np.int64)
        col_of = np.empty(N, np.int64)
        heap = [(0.0, s) for s in range(nslot)]
        heapq.heapify(heap)
        for n in order:
            while True:
                l, s = heapq.heappop(heap)
                if fill[s] < cap[s] and l == max(load_lo[s], load_hi[s]):
                    break
                if fill[s] < cap[s]:
                    heapq.heappush(heap, (max(load_lo[s], load_hi[s]), s))
            slot_of[n] = s
            col_of[n] = fill[s]
            fill[s] += 1
            # self-loop counts toward lo iff this slot's core is < NCORE/2
            if s // TILES < NCORE // 2:
                load_lo[s] += key_lo[n] + 1.0
                load_hi[s] += key_hi[n]
            else:
                load_lo[s] += key_lo[n]
                load_hi[s] += key_hi[n] + 1.0
            if fill[s] < cap[s]:
                heapq.heappush(heap, (max(load_lo[s], load_hi[s]), s))
        return slot_of, col_of

    # pass 1: split unknown -> assume half/half
    half = indeg.astype(np.float64) / 2.0
    slot_of, col_of = greedy(half, half)
    # pass 2: realized lo/hi per dst node under pass-1 cores
    core1 = slot_of // TILES
    src_hi = core1[src] >= NCORE // 2
    lo_cnt = np.bincount(dst[~src_hi], minlength=N).astype(np.float64)
    hi_cnt = np.bincount(dst[src_hi], minlength=N).astype(np.float64)
    slot_of, col_of = greedy(lo_cnt, hi_cnt)
    core_of = slot_of // TILES
    tile_of = slot_of % TILES
    loc_of = tile_of * 128 + col_of
    pid_of = core_of * NP + loc_of
    return src, dst, dinv, core_of, tile_of, col_of, loc_of, pid_of


def _build_streams(src, dst, core_of, tile_of, col_of, pid_of):
    """Per-core edge streams with uniform (tile, half) chunk counts."""
    N, TILES, HALF, BTILES = _CFG["N"], _CFG["TILES"], _CFG["HALF"], _CFG["BTILES"]
    nodes = np.arange(N, dtype=np.int64)
    a_src = np.concatenate([src, nodes])
    a_dst = np.concatenate([dst, nodes])

    e_core = core_of[a_dst]
    e_tile = tile_of[a_dst]
    e_dcol = col_of[a_dst]
    e_spid = pid_of[a_src]
    e_hi = (e_spid >= HALF).astype(np.int64)

    key = (e_core * TILES + e_tile) * 2 + e_hi
    cnt = np.bincount(key, minlength=NCORE * TILES * 2).reshape(NCORE, TILES, 2)
    K_lo = max(1, int(np.ceil(cnt[:, :, 0].max() / 128)))
    K_hi = max(1, int(np.ceil(cnt[:, :, 1].max() / 128)))

    order = np.lexsort((e_spid, e_hi, e_tile, e_core))
    e_core, e_tile, e_dcol, e_spid, e_hi = (
        e_core[order], e_tile[order], e_dcol[order], e_spid[order], e_hi[order])

    batches = []
    t = 0
    while t < TILES:
        batches.append(list(range(t, min(t + BTILES, TILES))))
        t += BTILES

    flat = cnt.reshape(-1)
    csum = np.concatenate([[0], np.cumsum(flat)])
    starts = csum[:-1].reshape(NCORE, TILES, 2)

    per_core = []
    for c in range(NCORE):
        idx_segs = []
        dst_cols = []
        for bt in batches:
            for h in range(2):
                K = K_lo if h == 0 else K_hi
                seg_idx = np.zeros((len(bt) * K * 128,), np.int64)
                seg_dst = np.full((len(bt) * K * 128,), -1.0, np.float32)
                for j, t in enumerate(bt):
                    s0 = starts[c, t, h]
                    n = cnt[c, t, h]
                    sl = slice(j * K * 128, j * K * 128 + n)
                    seg_idx[sl] = e_spid[s0 : s0 + n] - (HALF if h else 0)
                    seg_dst[sl] = e_dcol[s0 : s0 + n]
                w = seg_idx.astype(np.int16).reshape(-1, 16).T.copy()
                idx_segs.append(np.tile(w, (8, 1)))
                dst_cols.append(
                    seg_dst.reshape(-1, 128).T.copy().astype(np.float16))
        per_core.append((np.concatenate(idx_segs, axis=1),
                         np.concatenate(dst_cols, axis=1)))

    meta = dict(K_lo=K_lo, K_hi=K_hi, batches=batches)
    return per_core, meta


# ----------------------------------------------------------------------------
# device kernel
# ----------------------------------------------------------------------------

def _build_nc(meta, FT, CT):
    import concourse.bacc as bacc
    import concourse.mybir as mybir
    from concourse import tile

    N, TILES, NP = _CFG["N"], _CFG["TILES"], _CFG["NP"]
    NPAD, HALF, LASTV = _CFG["NPAD"], _CFG["HALF"], _CFG["LASTV"]
    F16, F32, I16 = mybir.dt.float16, mybir.dt.float32, mybir.dt.int16
    K_lo, K_hi, batches = meta["K_lo"], meta["K_hi"], meta["batches"]

    nc = bacc.Bacc()
    t_xT = nc.declare_dram_parameter("xT", [DIN, NP], F32, isOutput=False)
    t_idx = nc.declare_dram_parameter("idx", [128, FT], I16, isOutput=False)
    t_dst = nc.declare_dram_parameter("dst", [128, CT], F16, isOutput=False)
    t_iota = nc.declare_dram_parameter("iota", [128, 128], F16, isOutput=False)
    t_idn = nc.declare_dram_parameter("idn", [64, 64], F16, isOutput=False)
    t_dvr = nc.declare_dram_parameter("dvr", [64, NP], F16, isOutput=False)
    t_dvl = nc.declare_dram_parameter("dvl", [128, TILES], F32, isOutput=False)
    t_W1 = nc.declare_dram_parameter("W1", [DIN, DOUT], F32, isOutput=False)
    t_W2 = nc.declare_dram_parameter("W2", [DOUT, DOUT], F16, isOutput=False)
    t_W3 = nc.declare_dram_parameter("W3", [DOUT, DIN], F16, isOutput=False)
    t_p12 = nc.declare_dram_parameter("p12", [64, 4], F32, isOutput=False)
    t_p3 = nc.declare_dram_parameter("p3", [128, 4], F32, isOutput=False)
    o_out = nc.declare_dram_parameter("outT", [2, 128, NP], F32, isOutput=True)

    h_loc = [nc.dram_tensor(f"h_loc{l}", [NP, 2 * DOUT], F16) for l in range(3)]
    h_tab = [nc.dram_tensor(f"h_tab{l}", [NPAD, 2 * DOUT], F16, addr_space="Shared")
             for l in range(3)]
    strows = [64, 64, 128]
    st_in = [nc.dram_tensor(f"st_in{l}", [strows[l], 4], F32) for l in range(3)]
    st_out = [nc.dram_tensor(f"st_out{l}", [strows[l], 4], F32,
                             addr_space="Shared") for l in range(3)]

    CB = len(batches[0]) * (K_lo + K_hi)
    RG = [list(range(NCORE))]

    with tile.TileContext(nc) as tc:
        with (
            tc.tile_pool(name="const", bufs=1) as pc,
            tc.tile_pool(name="work", bufs=2) as pw,
            tc.tile_pool(name="gat", bufs=1) as pg,
            tc.tile_pool(name="psA", bufs=4, space="PSUM") as psA,
            tc.tile_pool(name="psC", bufs=2, space="PSUM") as psC,
            tc.tile_pool(name="psB", bufs=2, space="PSUM") as psB,
        ):
            # ---- persistent loads ----
            s_idx = pc.tile([128, FT], I16)
            nc.sync.dma_start(out=s_idx[:], in_=t_idx[:])
            s_dst = pc.tile([128, CT], F16)
            nc.sync.dma_start(out=s_dst[:], in_=t_dst[:])
            s_iota = pc.tile([128, 128], F16)
            nc.sync.dma_start(out=s_iota[:], in_=t_iota[:])
            s_idn = pc.tile([64, 64], F16)
            nc.sync.dma_start(out=s_idn[:], in_=t_idn[:])
            s_dvr = pc.tile([64, NP], F16)
            nc.sync.dma_start(out=s_dvr[:], in_=t_dvr[:])
            s_dvl = pc.tile([128, TILES], F32)
            nc.sync.dma_start(out=s_dvl[:], in_=t_dvl[:])
            s_W1 = pc.tile([128, 2, DOUT], F32)
            nc.sync.dma_start(
                out=s_W1[:], in_=t_W1[:].rearrange("(k p) f -> p k f", p=128))
            s_W2 = pc.tile([DOUT, DOUT], F16)
            nc.sync.dma_start(out=s_W2[:], in_=t_W2[:])
            s_W3 = pc.tile([DOUT, DIN], F16)
            nc.sync.dma_start(out=s_W3[:], in_=t_W3[:])
            s_p12 = pc.tile([64, 4], F32)
            nc.sync.dma_start(out=s_p12[:], in_=t_p12[:])
            s_p3 = pc.tile([128, 4], F32)
            nc.sync.dma_start(out=s_p3[:], in_=t_p3[:])

            s_aT = pc.tile([64, NP], F16)
            s_z3 = pc.tile([128, 2, NP], F32, tag="zz")
            s_z12 = pc.tile([64, NP], F32, tag="zz")
            s_hst = pc.tile([128, TILES, 2 * DOUT], F16)
            s_stat = pc.tile([128, 8], F32)
            nc.vector.memset(s_hst[:], 0.0)
            s_vec = pc.tile([128, 8], F32)

            def build_table(l):
                for t in range(TILES):
                    tr = slice(t * 128, (t + 1) * 128)
                    if l == 0:
                        ph = psB.tile([128, DOUT], F32, tag="ph")
                        xt = pw.tile([128, 2, 128], F32, tag="xt")
                        nc.sync.dma_start(
                            out=xt[:],
                            in_=t_xT[:, tr].rearrange("(k p) n -> p k n", p=128))
                        for k in range(2):
                            nc.tensor.matmul(
                                ph[:], xt[:, k, :], s_W1[:, k, :],
                                start=(k == 0), stop=(k == 1))
                    elif l == 1:
                        ph = psB.tile([128, DOUT], F32, tag="ph")
                        nc.tensor.matmul(
                            ph[:], s_aT[:, tr], s_W2[:], start=True, stop=True)
                    else:
                        ph = psB.tile([128, DOUT], F16, tag="ph")
                        nc.tensor.transpose(ph[:], s_aT[:, tr], s_idn[:])
                    nc.vector.tensor_scalar(
                        s_hst[:, t, 0:DOUT], ph[:], s_dvl[:, t:t + 1], None,
                        mybir.AluOpType.mult)
                nc.sync.dma_start(
                    out=h_loc[l][:].rearrange("(t p) f -> p t f", p=128),
                    in_=s_hst[:])
                nc.gpsimd.collective_compute(
                    "AllGather", mybir.AluOpType.bypass, replica_groups=RG,
                    ins=[h_loc[l][:].opt()], outs=[h_tab[l][:].opt()])

            def aggregate(l):
                cb0 = 0
                f0 = 0
                for bt in batches:
                    nb = len(bt)
                    n_lo, n_hi = nb * K_lo * 128, nb * K_hi * 128
                    CBb = nb * (K_lo + K_hi)
                    m16 = pw.tile([128, CB, 2 * DOUT], F16, tag="m16")
                    nc.gpsimd.dma_gather(
                        m16[:, 0:nb * K_lo, :], h_tab[l][0:HALF, :],
                        s_idx[:, f0:f0 + n_lo // 16], n_lo, n_lo, 2 * DOUT,
                        single_packet=False)
                    f0 += n_lo // 16
                    nc.gpsimd.dma_gather(
                        m16[:, nb * K_lo:CBb, :], h_tab[l][HALF:NPAD, :],
                        s_idx[:, f0:f0 + n_hi // 16], n_hi, n_hi, 2 * DOUT,
                        single_packet=False)
                    f0 += n_hi // 16
                    sS = pw.tile([128, CB, 128], F16, tag="sS")
                    nc.vector.tensor_tensor(
                        sS[:, 0:CBb, :],
                        s_iota[:].unsqueeze(1).broadcast_to([128, CBb, 128]),
                        s_dst[:, cb0:cb0 + CBb].unsqueeze(2).broadcast_to(
                            [128, CBb, 128]),
                        mybir.AluOpType.is_equal)
                    for j, t in enumerate(bt):
                        pa = psA.tile([64, 128], F32, tag="pa")
                        ck = (list(range(j * K_lo, (j + 1) * K_lo))
                              + list(range(nb * K_lo + j * K_hi,
                                           nb * K_lo + (j + 1) * K_hi)))
                        for i, c in enumerate(ck):
                            nc.tensor.matmul(
                                pa[:], m16[:, c, 0:DOUT], sS[:, c, :],
                                start=(i == 0), stop=(i == len(ck) - 1))
                        tr = slice(t * 128, (t + 1) * 128)
                        if l < 2:
                            nc.vector.tensor_tensor(
                                s_z12[:, tr], pa[:], s_dvr[:, tr],
                                mybir.AluOpType.mult)
                        else:
                            ag = pw.tile([64, 128], F16, tag="ag")
                            nc.vector.tensor_tensor(
                                ag[:], pa[:], s_dvr[:, tr], mybir.AluOpType.mult)
                            for hf in range(2):
                                p3p = psC.tile([128, 128], F32, tag="p3p")
                                nc.tensor.matmul(
                                    p3p[:], s_W3[:, hf * 128:(hf + 1) * 128],
                                    ag[:], start=True, stop=True)
                                nc.vector.tensor_copy(s_z3[:, hf, tr], p3p[:])
                    cb0 += CBb

            def bn_stats(l):
                nrows = strows[l]
                if l < 2:
                    pt = pw.tile([64, TILES, 2], F32, tag="pt")
                    sq = pw.tile([64, 128], F32, tag="sq")
                    for t in range(TILES):
                        v = LASTV if t == TILES - 1 else 128
                        zt = s_z12[:, t * 128: t * 128 + v]
                        nc.vector.reduce_sum(
                            pt[:, t, 0:1], zt, axis=mybir.AxisListType.X)
                        nc.vector.tensor_tensor(
                            sq[:, 0:v], zt, zt, mybir.AluOpType.mult)
                        nc.vector.reduce_sum(
                            pt[:, t, 1:2], sq[:, 0:v], axis=mybir.AxisListType.X)
                    nc.vector.reduce_sum(
                        s_stat[0:64, 0:1], pt[:, :, 0], axis=mybir.AxisListType.X)
                    nc.vector.reduce_sum(
                        s_stat[0:64, 1:2], pt[:, :, 1], axis=mybir.AxisListType.X)
                    nc.vector.memset(s_stat[0:64, 2:4], 0.0)
                else:
                    pt = pw.tile([128, TILES, 4], F32, tag="pt3")
                    sq = pw.tile([128, 128], F32, tag="sq3")
                    for t in range(TILES):
                        v = LASTV if t == TILES - 1 else 128
                        for hf in range(2):
                            zt = s_z3[:, hf, t * 128: t * 128 + v]
                            nc.vector.reduce_sum(
                                pt[:, t, 2 * hf:2 * hf + 1], zt,
                                axis=mybir.AxisListType.X)
                            nc.vector.tensor_tensor(
                                sq[:, 0:v], zt, zt, mybir.AluOpType.mult)
                            nc.vector.reduce_sum(
                                pt[:, t, 2 * hf + 1:2 * hf + 2], sq[:, 0:v],
                                axis=mybir.AxisListType.X)
                    for k in range(4):
                        nc.vector.reduce_sum(
                            s_stat[:, k:k + 1], pt[:, :, k],
                            axis=mybir.AxisListType.X)
                nc.sync.dma_start(out=st_in[l][:], in_=s_stat[0:nrows, 0:4])
                nc.gpsimd.collective_compute(
                    "AllReduce", mybir.AluOpType.add, replica_groups=RG,
                    ins=[st_in[l][:].opt()], outs=[st_out[l][:].opt()])
                nc.sync.dma_start(out=s_stat[0:nrows, 4:8], in_=st_out[l][:])
                invN = 1.0 / float(N)
                npair = 1 if l < 2 else 2
                for p in range(npair):
                    r = slice(0, nrows)
                    su = s_stat[r, 4 + 2 * p:5 + 2 * p]
                    s2 = s_stat[r, 5 + 2 * p:6 + 2 * p]
                    m = s_vec[r, 4:5]
                    nc.vector.tensor_scalar(m, su, invN, None, mybir.AluOpType.mult)
                    ex2 = s_vec[r, 5:6]
                    nc.vector.tensor_scalar(s2, s2, invN, None, mybir.AluOpType.mult)
                    nc.vector.tensor_tensor(ex2, m, m, mybir.AluOpType.mult)
                    nc.vector.tensor_tensor(ex2, s2, ex2, mybir.AluOpType.subtract)
                    sd = s_vec[r, 6:7]
                    nc.vector.tensor_scalar(ex2, ex2, float(EPS), None,
                                            mybir.AluOpType.add)
                    nc.scalar.activation(sd, ex2, mybir.ActivationFunctionType.Sqrt)
                    inv = s_vec[r, 7:8]
                    nc.vector.reciprocal(inv, sd)
                    if l < 2:
                        g = s_p12[:, 2 * l:2 * l + 1]
                        be = s_p12[:, 2 * l + 1:2 * l + 2]
                    else:
                        g = s_p3[:, p:p + 1]
                        be = s_p3[:, 2 + p:3 + p]
                    sc = s_vec[r, 2 * p:2 * p + 1]
                    sh = s_vec[r, 2 * p + 1:2 * p + 2]
                    nc.vector.tensor_tensor(sc, g, inv, mybir.AluOpType.mult)
                    nc.vector.tensor_tensor(sh, m, sc, mybir.AluOpType.mult)
                    nc.vector.tensor_tensor(sh, be, sh, mybir.AluOpType.subtract)

            stage = os.environ.get("GCN_STAGE", "full")
            if stage == "tableng":
                # table build without the collective (timing control)
                for t in range(TILES):
                    tr = slice(t * 128, (t + 1) * 128)
                    ph = psB.tile([128, DOUT], F32, tag="ph")
                    xt = pw.tile([128, 2, 128], F32, tag="xt")
                    nc.sync.dma_start(
                        out=xt[:],
                        in_=t_xT[:, tr].rearrange("(k p) n -> p k n", p=128))
                    for k in range(2):
                        nc.tensor.matmul(ph[:], xt[:, k, :], s_W1[:, k, :],
                                         start=(k == 0), stop=(k == 1))
                    nc.vector.tensor_scalar(
                        s_hst[:, t, 0:DOUT], ph[:], s_dvl[:, t:t + 1], None,
                        mybir.AluOpType.mult)
                nc.sync.dma_start(
                    out=h_loc[0][:].rearrange("(t p) f -> p t f", p=128),
                    in_=s_hst[:])
                nc.sync.dma_start(out=o_out[0][:, 0:TILES * DOUT],
                                  in_=s_hst[:].rearrange("p t f -> p (t f)"))
            elif stage == "gonly":
                # table + collective + gathers only (no cast/S/matmul)
                build_table(0)
                f0 = 0
                cb0 = 0
                for bt in batches:
                    nb = len(bt)
                    n_lo, n_hi = nb * K_lo * 128, nb * K_hi * 128
                    CBb = nb * (K_lo + K_hi)
                    m32 = pg.tile([128, CB, 2 * DOUT], F16, tag="m32")
                    nc.gpsimd.dma_gather(
                        m32[:, 0:nb * K_lo, :], h_tab[0][0:HALF, :],
                        s_idx[:, f0:f0 + n_lo // 16], n_lo, n_lo, 2 * DOUT,
                        single_packet=False)
                    f0 += n_lo // 16
                    nc.gpsimd.dma_gather(
                        m32[:, nb * K_lo:CBb, :], h_tab[0][HALF:NPAD, :],
                        s_idx[:, f0:f0 + n_hi // 16], n_hi, n_hi, 2 * DOUT,
                        single_packet=False)
                    f0 += n_hi // 16
                    cb0 += CBb
                nc.sync.dma_start(out=o_out[0][:, 0:TILES * DOUT],
                                  in_=s_hst[:].rearrange("p t f -> p (t f)"))
            elif stage == "table":
                build_table(0)
                nc.sync.dma_start(out=o_out[0][:, 0:TILES * DOUT],
                                  in_=s_hst[:].rearrange("p t f -> p (t f)"))
                nc.vector.memset(s_z3[:, 1, 0:128], 0.0)
                nc.sync.dma_start(out=o_out[1][:, 0:128], in_=s_z3[:, 1, 0:128])
            elif stage == "agg":
                build_table(0)
                aggregate(0)
                nc.sync.dma_start(out=o_out[0][0:64, :], in_=s_z12[:, :])
                nc.sync.dma_start(out=o_out[1][0:64, :], in_=s_z12[:, :])
            if stage == "full":
                for l in range(2):
                    build_table(l)
                    aggregate(l)
                    bn_stats(l)
                    for t in range(TILES):
                        tr = slice(t * 128, (t + 1) * 128)
                        nc.scalar.activation(
                            s_aT[:, tr], s_z12[:, tr],
                            mybir.ActivationFunctionType.Relu,
                            bias=s_vec[0:64, 1:2], scale=s_vec[0:64, 0:1])
                build_table(2)
                aggregate(2)
                bn_stats(2)
                for t in range(TILES):
                    tr = slice(t * 128, (t + 1) * 128)
                    for hf in range(2):
                        z = s_z3[:, hf, tr]
                        nc.scalar.activation(
                            z, z, mybir.ActivationFunctionType.Identity,
                            bias=s_vec[:, 2 * hf + 1:2 * hf + 2],
                            scale=s_vec[:, 2 * hf:2 * hf + 1])
                        xt = pw.tile([128, 128], F32, tag="xr")
                        nc.sync.dma_start(
                            out=xt[:], in_=t_xT[hf * 128:(hf + 1) * 128, tr])
                        nc.vector.tensor_tensor(z, z, xt[:], mybir.AluOpType.add)
                        nc.vector.tensor_scalar(z, z, 0.0, None,
                                                mybir.AluOpType.max)
                for hf in range(2):
                    nc.sync.dma_start(out=o_out[hf], in_=s_z3[:, hf, :])

    nc.finalize()
    return nc


# ----------------------------------------------------------------------------
# entry point
# ----------------------------------------------------------------------------

def _prepare(x, ei, W1, g1, be1, W2, g2, be2, W3, g3, be3):
    N, NP, TILES = _CFG["N"], _CFG["NP"], _CFG["TILES"]
    x = np.asarray(x, np.float32)
    ei = np.asarray(ei, np.int32)
    src, dst, dinv, core_of, tile_of, col_of, loc_of, pid_of = _prep_graph(ei)
    per_core, meta = _build_streams(src, dst, core_of, tile_of, col_of, pid_of)

    iota = np.tile(np.arange(128, dtype=np.float16)[None, :], (128, 1))
    idn = np.eye(64, dtype=np.float16)
    p12 = np.stack([np.asarray(g1), np.asarray(be1),
                    np.asarray(g2), np.asarray(be2)], axis=1).astype(np.float32)
    g3c = np.asarray(g3, np.float32).reshape(2, 128).T
    be3c = np.asarray(be3, np.float32).reshape(2, 128).T
    p3 = np.concatenate([g3c, be3c], axis=1).astype(np.float32)

    in_maps = []
    for c in range(NCORE):
        nodes_c = np.nonzero(core_of == c)[0]
        lidx = loc_of[nodes_c]
        xT = np.zeros((DIN, NP), np.float32)
        xT[:, lidx] = x[nodes_c].T
        dvr = np.zeros((NP,), np.float32)
        dvr[lidx] = dinv[nodes_c]
        dvl = dvr.reshape(TILES, 128).T.copy()
        idx_all, dst_all = per_core[c]
        in_maps.append({
            "xT": xT, "idx": np.ascontiguousarray(idx_all),
            "dst": np.ascontiguousarray(dst_all), "iota": iota, "idn": idn,
            "dvr": np.tile(dvr[None, :], (64, 1)).astype(np.float16),
            "dvl": np.ascontiguousarray(dvl),
            "W1": np.asarray(W1, np.float32),
            "W2": np.asarray(W2, np.float32).astype(np.float16),
            "W3": np.asarray(W3, np.float32).astype(np.float16),
            "p12": p12, "p3": p3,
        })
    return in_maps, meta, core_of, loc_of


def kernel(x, ei, batch, W1, b1, g1, be1, W2, b2, g2, be2, W3, b3, g3, be3):
    global LAST_EXEC_NS
    from concourse.bass_utils import run_bass_kernel_spmd

    N, NP = _CFG["N"], _CFG["NP"]
    in_maps, meta, core_of, loc_of = _prepare(
        x, ei, W1, g1, be1, W2, g2, be2, W3, g3, be3)
    nc = _build_nc(meta, in_maps[0]["idx"].shape[1], in_maps[0]["dst"].shape[1])

    trace = bool(int(os.environ.get("GCN_TRACE", "0")))
    res = run_bass_kernel_spmd(nc, in_maps, list(range(NCORE)), trace=trace)
    if res.exec_time_ns is not None:
        LAST_EXEC_NS = res.exec_time_ns

    out = np.empty((N, DIN), np.float32)
    for c in range(NCORE):
        nodes_c = np.nonzero(core_of == c)[0]
        arr = res.results[c]["outT"].reshape(DIN, NP)
        out[nodes_c] = arr[:, loc_of[nodes_c]].T
    return out



# revision 10
# speedup vs baseline: 1.8277x; 1.8277x over previous
"""Trainium2 Bass kernel for a 3-layer GCN bottleneck block (50k nodes, 800k edges).

Strategy (8 NeuronCores, dst-node sharding):
- Host: relabel nodes into 8 cores x TILES tiles x 128 slots, balancing per-tile
  in-degree. Edges sorted by (dst tile, src half, src id); each (tile, half)
  group padded to a uniform chunk count so one SPMD program serves all cores.
  Self-loops become plain edges.
- All three convs aggregate in 64-dim space (conv3 rewritten as (A~ @ a2) @ W3).
- Per layer: compute local h-shard = a @ W with dinv[src] folded in, AllGather
  the [NPAD, 64] fp32 table, dma_gather 256B rows per edge (src-sorted for HBM
  locality), segment-sum via one-hot matmul: aggT[64f,128d] += M16.T @ S where
  S = is_equal(iota, dstloc) built in one DVE pass per batch (pads get dstloc=-1
  so their S column is zero). dinv[dst] applied during PSUM eviction.
- BatchNorm: per-core feature-major partial sums, tiny AllReduce, ACT-fused
  scale/bias/relu. Final layer: W3 matmul to 256-dim, BN3, residual, relu,
  output transposed; host untransposes/unpermutes.
"""

import os
import numpy as np

DIN = 256
DOUT = 64
EPS = 1e-5
NCORE = 8

# default (real-problem) geometry; test_sim.py overrides via configure()
_CFG = {}


def configure(N, E, NLOC, BTILES):
    TILES = (NLOC + 127) // 128
    NP = TILES * 128
    _CFG.update(
        N=N, E=E, NLOC=NLOC, TILES=TILES, NP=NP,
        NPAD=NCORE * NP, HALF=(NCORE * NP) // 2,
        LASTV=NLOC - 128 * (TILES - 1),       # valid rows in last tile
        BTILES=BTILES,
    )


configure(N=50000, E=800000, NLOC=6250, BTILES=3)

LAST_EXEC_NS = None    # set by kernel() when GCN_TRACE=1


# ----------------------------------------------------------------------------
# host-side graph prep
# ----------------------------------------------------------------------------

def _prep_graph(ei):
    import heapq
    N, TILES, NP = _CFG["N"], _CFG["TILES"], _CFG["NP"]
    LASTV = _CFG["LASTV"]
    src, dst = ei[0].astype(np.int64), ei[1].astype(np.int64)
    deg = np.bincount(dst, minlength=N).astype(np.float32) + 1.0
    dinv = (1.0 / np.sqrt(deg)).astype(np.float32)
    indeg = np.bincount(dst, minlength=N)

    # Balance BOTH lo and hi in-edge loads per (core,tile) slot: the padded
    # chunk counts K_lo/K_hi are set by the max over slots, so minimize the
    # larger of the two running loads. lo/hi of an edge depends on the SOURCE
    # node's final core, unknown during assignment — approximate with a
    # first-pass assignment by total degree, then rebalance on realized lo/hi.
    nslot = NCORE * TILES
    cap = np.full(nslot, 128, np.int64)
    cap[TILES - 1 :: TILES] = LASTV

    def greedy(key_lo, key_hi):
        order = np.argsort(-(key_lo + key_hi), kind="stable")
        load_lo = np.zeros(nslot, np.float64)
        load_hi = np.zeros(nslot, np.float64)
        fill = np.zeros(nslot, np.int64)
        slot_of = np.empty(N, np.int64)
        col_of = np.empty(N, np.int64)
        heap = [(0.0, s) for s in range(nslot)]
        heapq.heapify(heap)
        for n in order:
            while True:
                l, s = heapq.heappop(heap)
                if fill[s] < cap[s] and l == max(load_lo[s], load_hi[s]):
                    break
                if fill[s] < cap[s]:
                    heapq.heappush(heap, (max(load_lo[s], load_hi[s]), s))
            slot_of[n] = s
            col_of[n] = fill[s]
            fill[s] += 1
            # self-loop counts toward lo iff this slot's core is < NCORE/2
            # (self-loops handled on-device as a PE transpose when GCN_NOSELF)
            sl_add = 0.0 if os.environ.get("GCN_NOSELF") == "1" else 1.0
            if s // TILES < NCORE // 2:
                load_lo[s] += key_lo[n] + sl_add
                load_hi[s] += key_hi[n]
            else:
                load_lo[s] += key_lo[n]
                load_hi[s] += key_hi[n] + sl_add
            if fill[s] < cap[s]:
                heapq.heappush(heap, (max(load_lo[s], load_hi[s]), s))
        return slot_of, col_of

    # pass 1: split unknown -> assume half/half
    half = indeg.astype(np.float64) / 2.0
    slot_of, col_of = greedy(half, half)
    # pass 2: realized lo/hi per dst node under pass-1 cores
    core1 = slot_of // TILES
    src_hi = core1[src] >= NCORE // 2
    lo_cnt = np.bincount(dst[~src_hi], minlength=N).astype(np.float64)
    hi_cnt = np.bincount(dst[src_hi], minlength=N).astype(np.float64)
    slot_of, col_of = greedy(lo_cnt, hi_cnt)
    core_of = slot_of // TILES
    tile_of = slot_of % TILES
    loc_of = tile_of * 128 + col_of
    pid_of = core_of * NP + loc_of
    return src, dst, dinv, core_of, tile_of, col_of, loc_of, pid_of


def _build_streams(src, dst, core_of, tile_of, col_of, pid_of):
    """Per-core edge streams with uniform (tile, half) chunk counts."""
    N, TILES, HALF, BTILES = _CFG["N"], _CFG["TILES"], _CFG["HALF"], _CFG["BTILES"]
    if os.environ.get("GCN_NOSELF") == "1":
        a_src, a_dst = src, dst
    else:
        nodes = np.arange(N, dtype=np.int64)
        a_src = np.concatenate([src, nodes])
        a_dst = np.concatenate([dst, nodes])

    e_core = core_of[a_dst]
    e_tile = tile_of[a_dst]
    e_dcol = col_of[a_dst]
    e_spid = pid_of[a_src]
    e_hi = (e_spid >= HALF).astype(np.int64)

    key = (e_core * TILES + e_tile) * 2 + e_hi
    cnt = np.bincount(key, minlength=NCORE * TILES * 2).reshape(NCORE, TILES, 2)
    K_lo = max(1, int(np.ceil(cnt[:, :, 0].max() / 128)))
    K_hi = max(1, int(np.ceil(cnt[:, :, 1].max() / 128)))

    order = np.lexsort((e_spid, e_hi, e_tile, e_core))
    e_core, e_tile, e_dcol, e_spid, e_hi = (
        e_core[order], e_tile[order], e_dcol[order], e_spid[order], e_hi[order])

    batches = []
    t = 0
    while t < TILES:
        batches.append(list(range(t, min(t + BTILES, TILES))))
        t += BTILES

    flat = cnt.reshape(-1)
    csum = np.concatenate([[0], np.cumsum(flat)])
    starts = csum[:-1].reshape(NCORE, TILES, 2)

    per_core = []
    for c in range(NCORE):
        idx_segs = []
        dst_cols = []
        for bt in batches:
            for h in range(2):
                K = K_lo if h == 0 else K_hi
                seg_idx = np.zeros((len(bt) * K * 128,), np.int64)
                seg_dst = np.full((len(bt) * K * 128,), -1.0, np.float32)
                for j, t in enumerate(bt):
                    s0 = starts[c, t, h]
                    n = cnt[c, t, h]
                    sl = slice(j * K * 128, j * K * 128 + n)
                    seg_idx[sl] = e_spid[s0 : s0 + n] - (HALF if h else 0)
                    seg_dst[sl] = e_dcol[s0 : s0 + n]
                    if os.environ.get("GCN_SHUF") == "1":
                        rng = np.random.default_rng(12345 + c * 1000 + t * 2 + h)
                        perm = rng.permutation(n) + j * K * 128
                        seg_idx[sl] = seg_idx[perm]
                        seg_dst[sl] = seg_dst[perm]
                w = seg_idx.astype(np.int16).reshape(-1, 16).T.copy()
                idx_segs.append(np.tile(w, (8, 1)))
                dst_cols.append(
                    seg_dst.reshape(-1, 128).T.copy().astype(np.float16))
        per_core.append((np.concatenate(idx_segs, axis=1),
                         np.concatenate(dst_cols, axis=1)))

    meta = dict(K_lo=K_lo, K_hi=K_hi, batches=batches)
    return per_core, meta


# ----------------------------------------------------------------------------
# device kernel
# ----------------------------------------------------------------------------

def _build_nc(meta, FT, CT):
    import concourse.bacc as bacc
    import concourse.mybir as mybir
    from concourse import tile

    N, TILES, NP = _CFG["N"], _CFG["TILES"], _CFG["NP"]
    NPAD, HALF, LASTV = _CFG["NPAD"], _CFG["HALF"], _CFG["LASTV"]
    F16, F32, I16 = mybir.dt.float16, mybir.dt.float32, mybir.dt.int16
    K_lo, K_hi, batches = meta["K_lo"], meta["K_hi"], meta["batches"]

    NQ = int(os.environ.get("GCN_QUEUES", "1"))
    nc = bacc.Bacc(num_swdge_queues=NQ) if NQ > 1 else bacc.Bacc()
    qct = [0]

    def next_q():
        q = qct[0] % NQ
        qct[0] += 1
        return q

    t_xT = nc.declare_dram_parameter("xT", [DIN, NP], F32, isOutput=False)
    t_idx = nc.declare_dram_parameter("idx", [128, FT], I16, isOutput=False)
    t_dst = nc.declare_dram_parameter("dst", [128, CT], F16, isOutput=False)
    t_iota = nc.declare_dram_parameter("iota", [128, 128], F16, isOutput=False)
    t_idn = nc.declare_dram_parameter("idn", [64, 64], F16, isOutput=False)
    t_id2 = nc.declare_dram_parameter("id2", [128, 128], F16, isOutput=False)
    t_dvr = nc.declare_dram_parameter("dvr", [64, NP], F16, isOutput=False)
    t_dvl = nc.declare_dram_parameter("dvl", [128, TILES], F32, isOutput=False)
    t_W1 = nc.declare_dram_parameter("W1", [DIN, DOUT], F32, isOutput=False)
    t_W2 = nc.declare_dram_parameter("W2", [DOUT, DOUT], F16, isOutput=False)
    t_W3 = nc.declare_dram_parameter("W3", [DOUT, DIN], F16, isOutput=False)
    t_p12 = nc.declare_dram_parameter("p12", [64, 4], F32, isOutput=False)
    t_p3 = nc.declare_dram_parameter("p3", [128, 4], F32, isOutput=False)
    o_out = nc.declare_dram_parameter("outT", [2, 128, NP], F32, isOutput=True)

    h_loc = [nc.dram_tensor(f"h_loc{l}", [NP, 2 * DOUT], F16) for l in range(3)]
    h_tab = [nc.dram_tensor(f"h_tab{l}", [NPAD, 2 * DOUT], F16, addr_space="Shared")
             for l in range(3)]
    strows = [64, 64, 128]
    st_in = [nc.dram_tensor(f"st_in{l}", [strows[l], 4], F32) for l in range(3)]
    st_out = [nc.dram_tensor(f"st_out{l}", [strows[l], 4], F32,
                             addr_space="Shared") for l in range(3)]

    CB = len(batches[0]) * (K_lo + K_hi)
    RG = [list(range(NCORE))]

    with tile.TileContext(nc) as tc:
        NBUF = int(os.environ.get("GCN_BUFS", "2"))
        with (
            tc.tile_pool(name="const", bufs=1) as pc,
            tc.tile_pool(name="work", bufs=2) as pw,
            tc.tile_pool(name="m16p", bufs=NBUF) as pm,
            tc.tile_pool(name="gat", bufs=1) as pg,
            tc.tile_pool(name="psA", bufs=4, space="PSUM") as psA,
            tc.tile_pool(name="psC", bufs=2, space="PSUM") as psC,
            tc.tile_pool(name="psB", bufs=2, space="PSUM") as psB,
        ):
            # ---- persistent loads ----
            s_idx = pc.tile([128, FT], I16)
            nc.sync.dma_start(out=s_idx[:], in_=t_idx[:])
            s_dst = pc.tile([128, CT], F16)
            nc.sync.dma_start(out=s_dst[:], in_=t_dst[:])
            s_iota = pc.tile([128, 128], F16)
            nc.sync.dma_start(out=s_iota[:], in_=t_iota[:])
            s_idn = pc.tile([64, 64], F16)
            nc.sync.dma_start(out=s_idn[:], in_=t_idn[:])
            s_id2 = pc.tile([128, 128], F16)
            nc.sync.dma_start(out=s_id2[:], in_=t_id2[:])
            s_dvr = pc.tile([64, NP], F16)
            nc.sync.dma_start(out=s_dvr[:], in_=t_dvr[:])
            s_dvl = pc.tile([128, TILES], F32)
            nc.sync.dma_start(out=s_dvl[:], in_=t_dvl[:])
            s_W1 = pc.tile([128, 2, DOUT], F32)
            nc.sync.dma_start(
                out=s_W1[:], in_=t_W1[:].rearrange("(k p) f -> p k f", p=128))
            s_W2 = pc.tile([DOUT, DOUT], F16)
            nc.sync.dma_start(out=s_W2[:], in_=t_W2[:])
            s_W3 = pc.tile([DOUT, DIN], F16)
            nc.sync.dma_start(out=s_W3[:], in_=t_W3[:])
            s_p12 = pc.tile([64, 4], F32)
            nc.sync.dma_start(out=s_p12[:], in_=t_p12[:])
            s_p3 = pc.tile([128, 4], F32)
            nc.sync.dma_start(out=s_p3[:], in_=t_p3[:])

            s_aT = pc.tile([64, NP], F16)
            s_z3 = pc.tile([128, 2, NP], F32, tag="zz")
            s_z12 = pc.tile([64, NP], F32, tag="zz")
            s_hst = pc.tile([128, TILES, 2 * DOUT], F16)
            s_stat = pc.tile([128, 8], F32)
            nc.vector.memset(s_hst[:], 0.0)
            s_vec = pc.tile([128, 8], F32)

            def build_table(l):
                for t in range(TILES):
                    tr = slice(t * 128, (t + 1) * 128)
                    if l == 0:
                        ph = psB.tile([128, DOUT], F32, tag="ph")
                        xt = pw.tile([128, 2, 128], F32, tag="xt")
                        nc.sync.dma_start(
                            out=xt[:],
                            in_=t_xT[:, tr].rearrange("(k p) n -> p k n", p=128))
                        for k in range(2):
                            nc.tensor.matmul(
                                ph[:], xt[:, k, :], s_W1[:, k, :],
                                start=(k == 0), stop=(k == 1))
                    elif l == 1:
                        ph = psB.tile([128, DOUT], F32, tag="ph")
                        nc.tensor.matmul(
                            ph[:], s_aT[:, tr], s_W2[:], start=True, stop=True)
                    else:
                        ph = psB.tile([128, DOUT], F16, tag="ph")
                        nc.tensor.transpose(ph[:], s_aT[:, tr], s_idn[:])
                    nc.vector.tensor_scalar(
                        s_hst[:, t, 0:DOUT], ph[:], s_dvl[:, t:t + 1], None,
                        mybir.AluOpType.mult)
                nc.sync.dma_start(
                    out=h_loc[l][:].rearrange("(t p) f -> p t f", p=128),
                    in_=s_hst[:])
                nc.gpsimd.collective_compute(
                    "AllGather", mybir.AluOpType.bypass, replica_groups=RG,
                    ins=[h_loc[l][:].opt()], outs=[h_tab[l][:].opt()])

            def aggregate(l):
                cb0 = 0
                f0 = 0
                for bt in batches:
                    nb = len(bt)
                    n_lo, n_hi = nb * K_lo * 128, nb * K_hi * 128
                    CBb = nb * (K_lo + K_hi)
                    m16 = pm.tile([128, CB, 2 * DOUT], F16, tag="m16")
                    nc.gpsimd.dma_gather(
                        m16[:, 0:nb * K_lo, :], h_tab[l][0:HALF, :],
                        s_idx[:, f0:f0 + n_lo // 16], n_lo, n_lo, 2 * DOUT,
                        single_packet=False, queue_num=next_q())
                    f0 += n_lo // 16
                    nc.gpsimd.dma_gather(
                        m16[:, nb * K_lo:CBb, :], h_tab[l][HALF:NPAD, :],
                        s_idx[:, f0:f0 + n_hi // 16], n_hi, n_hi, 2 * DOUT,
                        single_packet=False, queue_num=next_q())
                    f0 += n_hi // 16
                    sS = pw.tile([128, CB, 128], F16, tag="sS")
                    nc.vector.tensor_tensor(
                        sS[:, 0:CBb, :],
                        s_iota[:].unsqueeze(1).broadcast_to([128, CBb, 128]),
                        s_dst[:, cb0:cb0 + CBb].unsqueeze(2).broadcast_to(
                            [128, CBb, 128]),
                        mybir.AluOpType.is_equal)
                    for j, t in enumerate(bt):
                        pa = psA.tile([64, 128], F32, tag="pa")
                        ck = (list(range(j * K_lo, (j + 1) * K_lo))
                              + list(range(nb * K_lo + j * K_hi,
                                           nb * K_lo + (j + 1) * K_hi)))
                        noself = os.environ.get("GCN_NOSELF") == "1"
                        if noself:
                            # self-loop term: pa starts as local-table tile
                            # transposed ([128 nodes,64f] -> [64f,128 cols])
                            nc.tensor.matmul(
                                pa[:], s_hst[:, t, 0:DOUT], s_id2[:],
                                start=True, stop=False)
                        for i, c in enumerate(ck):
                            nc.tensor.matmul(
                                pa[:], m16[:, c, 0:DOUT], sS[:, c, :],
                                start=(i == 0 and not noself),
                                stop=(i == len(ck) - 1))
                        tr = slice(t * 128, (t + 1) * 128)
                        if l < 2:
                            nc.vector.tensor_tensor(
                                s_z12[:, tr], pa[:], s_dvr[:, tr],
                                mybir.AluOpType.mult)
                        else:
                            ag = pw.tile([64, 128], F16, tag="ag")
                            nc.vector.tensor_tensor(
                                ag[:], pa[:], s_dvr[:, tr], mybir.AluOpType.mult)
                            for hf in range(2):
                                p3p = psC.tile([128, 128], F32, tag="p3p")
                                nc.tensor.matmul(
                                    p3p[:], s_W3[:, hf * 128:(hf + 1) * 128],
                                    ag[:], start=True, stop=True)
                                nc.vector.tensor_copy(s_z3[:, hf, tr], p3p[:])
                    cb0 += CBb

            def bn_stats(l):
                nrows = strows[l]
                if l < 2:
                    pt = pw.tile([64, TILES, 2], F32, tag="pt")
                    sq = pw.tile([64, 128], F32, tag="sq")
                    for t in range(TILES):
                        v = LASTV if t == TILES - 1 else 128
                        zt = s_z12[:, t * 128: t * 128 + v]
                        nc.vector.reduce_sum(
                            pt[:, t, 0:1], zt, axis=mybir.AxisListType.X)
                        nc.vector.tensor_tensor(
                            sq[:, 0:v], zt, zt, mybir.AluOpType.mult)
                        nc.vector.reduce_sum(
                            pt[:, t, 1:2], sq[:, 0:v], axis=mybir.AxisListType.X)
                    nc.vector.reduce_sum(
                        s_stat[0:64, 0:1], pt[:, :, 0], axis=mybir.AxisListType.X)
                    nc.vector.reduce_sum(
                        s_stat[0:64, 1:2], pt[:, :, 1], axis=mybir.AxisListType.X)
                    nc.vector.memset(s_stat[0:64, 2:4], 0.0)
                else:
                    pt = pw.tile([128, TILES, 4], F32, tag="pt3")
                    sq = pw.tile([128, 128], F32, tag="sq3")
                    for t in range(TILES):
                        v = LASTV if t == TILES - 1 else 128
                        for hf in range(2):
                            zt = s_z3[:, hf, t * 128: t * 128 + v]
                            nc.vector.reduce_sum(
                                pt[:, t, 2 * hf:2 * hf + 1], zt,
                                axis=mybir.AxisListType.X)
                            nc.vector.tensor_tensor(
                                sq[:, 0:v], zt, zt, mybir.AluOpType.mult)
                            nc.vector.reduce_sum(
                                pt[:, t, 2 * hf + 1:2 * hf + 2], sq[:, 0:v],
                                axis=mybir.AxisListType.X)
                    for k in range(4):
                        nc.vector.reduce_sum(
                            s_stat[:, k:k + 1], pt[:, :, k],
                            axis=mybir.AxisListType.X)
                nc.sync.dma_start(out=st_in[l][:], in_=s_stat[0:nrows, 0:4])
                nc.gpsimd.collective_compute(
                    "AllReduce", mybir.AluOpType.add, replica_groups=RG,
                    ins=[st_in[l][:].opt()], outs=[st_out[l][:].opt()])
                nc.sync.dma_start(out=s_stat[0:nrows, 4:8], in_=st_out[l][:])
                invN = 1.0 / float(N)
                npair = 1 if l < 2 else 2
                for p in range(npair):
                    r = slice(0, nrows)
                    su = s_stat[r, 4 + 2 * p:5 + 2 * p]
                    s2 = s_stat[r, 5 + 2 * p:6 + 2 * p]
                    m = s_vec[r, 4:5]
                    nc.vector.tensor_scalar(m, su, invN, None, mybir.AluOpType.mult)
                    ex2 = s_vec[r, 5:6]
                    nc.vector.tensor_scalar(s2, s2, invN, None, mybir.AluOpType.mult)
                    nc.vector.tensor_tensor(ex2, m, m, mybir.AluOpType.mult)
                    nc.vector.tensor_tensor(ex2, s2, ex2, mybir.AluOpType.subtract)
                    sd = s_vec[r, 6:7]
                    nc.vector.tensor_scalar(ex2, ex2, float(EPS), None,
                                            mybir.AluOpType.add)
                    nc.scalar.activation(sd, ex2, mybir.ActivationFunctionType.Sqrt)
                    inv = s_vec[r, 7:8]
                    nc.vector.reciprocal(inv, sd)
                    if l < 2:
                        g = s_p12[:, 2 * l:2 * l + 1]
                        be = s_p12[:, 2 * l + 1:2 * l + 2]
                    else:
                        g = s_p3[:, p:p + 1]
                        be = s_p3[:, 2 + p:3 + p]
                    sc = s_vec[r, 2 * p:2 * p + 1]
                    sh = s_vec[r, 2 * p + 1:2 * p + 2]
                    nc.vector.tensor_tensor(sc, g, inv, mybir.AluOpType.mult)
                    nc.vector.tensor_tensor(sh, m, sc, mybir.AluOpType.mult)
                    nc.vector.tensor_tensor(sh, be, sh, mybir.AluOpType.subtract)

            stage = os.environ.get("GCN_STAGE", "full")
            if stage == "nop":
                nc.sync.dma_start(out=o_out[0][:, 0:4], in_=s_dvl[:, 0:4])
            elif stage == "tableng":
                # table build without the collective (timing control)
                for t in range(TILES):
                    tr = slice(t * 128, (t + 1) * 128)
                    ph = psB.tile([128, DOUT], F32, tag="ph")
                    xt = pw.tile([128, 2, 128], F32, tag="xt")
                    nc.sync.dma_start(
                        out=xt[:],
                        in_=t_xT[:, tr].rearrange("(k p) n -> p k n", p=128))
                    for k in range(2):
                        nc.tensor.matmul(ph[:], xt[:, k, :], s_W1[:, k, :],
                                         start=(k == 0), stop=(k == 1))
                    nc.vector.tensor_scalar(
                        s_hst[:, t, 0:DOUT], ph[:], s_dvl[:, t:t + 1], None,
                        mybir.AluOpType.mult)
                nc.sync.dma_start(
                    out=h_loc[0][:].rearrange("(t p) f -> p t f", p=128),
                    in_=s_hst[:])
                nc.gpsimd.dma_start(out=o_out[0][:, 0:TILES * 2 * DOUT],
                                    in_=s_hst[:].rearrange("p t f -> p (t f)"))
            elif stage == "gonly":
                # table + collective + gathers only (no cast/S/matmul)
                build_table(0)
                f0 = 0
                cb0 = 0
                for bt in batches:
                    nb = len(bt)
                    n_lo, n_hi = nb * K_lo * 128, nb * K_hi * 128
                    CBb = nb * (K_lo + K_hi)
                    m32 = pg.tile([128, CB, 2 * DOUT], F16, tag="m32")
                    nc.gpsimd.dma_gather(
                        m32[:, 0:nb * K_lo, :], h_tab[0][0:HALF, :],
                        s_idx[:, f0:f0 + n_lo // 16], n_lo, n_lo, 2 * DOUT,
                        single_packet=False, queue_num=next_q())
                    f0 += n_lo // 16
                    nc.gpsimd.dma_gather(
                        m32[:, nb * K_lo:CBb, :], h_tab[0][HALF:NPAD, :],
                        s_idx[:, f0:f0 + n_hi // 16], n_hi, n_hi, 2 * DOUT,
                        single_packet=False, queue_num=next_q())
                    f0 += n_hi // 16
                    cb0 += CBb
                nc.gpsimd.dma_start(out=o_out[0][:, 0:TILES * 2 * DOUT],
                                    in_=s_hst[:].rearrange("p t f -> p (t f)"))
            elif stage == "table":
                build_table(0)
                nc.gpsimd.dma_start(out=o_out[0][:, 0:TILES * 2 * DOUT],
                                    in_=s_hst[:].rearrange("p t f -> p (t f)"))
                nc.vector.memset(s_z3[:, 1, 0:128], 0.0)
                nc.sync.dma_start(out=o_out[1][:, 0:128], in_=s_z3[:, 1, 0:128])
            elif stage == "agg":
                build_table(0)
                aggregate(0)
                nc.sync.dma_start(out=o_out[0][0:64, :], in_=s_z12[:, :])
                nc.sync.dma_start(out=o_out[1][0:64, :], in_=s_z12[:, :])
            if stage == "full":
                for l in range(2):
                    build_table(l)
                    aggregate(l)
                    bn_stats(l)
                    for t in range(TILES):
                        tr = slice(t * 128, (t + 1) * 128)
                        nc.scalar.activation(
                            s_aT[:, tr], s_z12[:, tr],
                            mybir.ActivationFunctionType.Relu,
                            bias=s_vec[0:64, 1:2], scale=s_vec[0:64, 0:1])
                build_table(2)
                aggregate(2)
                bn_stats(2)
                for t in range(TILES):
                    tr = slice(t * 128, (t + 1) * 128)
                    for hf in range(2):
                        z = s_z3[:, hf, tr]
                        nc.scalar.activation(
                            z, z, mybir.ActivationFunctionType.Identity,
                            bias=s_vec[:, 2 * hf + 1:2 * hf + 2],
                            scale=s_vec[:, 2 * hf:2 * hf + 1])
                        xt = pw.tile([128, 128], F32, tag="xr")
                        nc.sync.dma_start(
                            out=xt[:], in_=t_xT[hf * 128:(hf + 1) * 128, tr])
                        nc.vector.tensor_tensor(z, z, xt[:], mybir.AluOpType.add)
                        nc.vector.tensor_scalar(z, z, 0.0, None,
                                                mybir.AluOpType.max)
                for hf in range(2):
                    nc.sync.dma_start(out=o_out[hf], in_=s_z3[:, hf, :])

    nc.finalize()
    return nc


# ----------------------------------------------------------------------------
# entry point
# ----------------------------------------------------------------------------

def _prepare(x, ei, W1, g1, be1, W2, g2, be2, W3, g3, be3):
    N, NP, TILES = _CFG["N"], _CFG["NP"], _CFG["TILES"]
    x = np.asarray(x, np.float32)
    ei = np.asarray(ei, np.int32)
    src, dst, dinv, core_of, tile_of, col_of, loc_of, pid_of = _prep_graph(ei)
    per_core, meta = _build_streams(src, dst, core_of, tile_of, col_of, pid_of)

    iota = np.tile(np.arange(128, dtype=np.float16)[None, :], (128, 1))
    idn = np.eye(64, dtype=np.float16)
    id2 = np.eye(128, dtype=np.float16)
    p12 = np.stack([np.asarray(g1), np.asarray(be1),
                    np.asarray(g2), np.asarray(be2)], axis=1).astype(np.float32)
    g3c = np.asarray(g3, np.float32).reshape(2, 128).T
    be3c = np.asarray(be3, np.float32).reshape(2, 128).T
    p3 = np.concatenate([g3c, be3c], axis=1).astype(np.float32)

    in_maps = []
    for c in range(NCORE):
        nodes_c = np.nonzero(core_of == c)[0]
        lidx = loc_of[nodes_c]
        xT = np.zeros((DIN, NP), np.float32)
        xT[:, lidx] = x[nodes_c].T
        dvr = np.zeros((NP,), np.float32)
        dvr[lidx] = dinv[nodes_c]
        dvl = dvr.reshape(TILES, 128).T.copy()
        idx_all, dst_all = per_core[c]
        in_maps.append({
            "xT": xT, "idx": np.ascontiguousarray(idx_all),
            "dst": np.ascontiguousarray(dst_all), "iota": iota, "idn": idn,
            "id2": id2,
            "dvr": np.tile(dvr[None, :], (64, 1)).astype(np.float16),
            "dvl": np.ascontiguousarray(dvl),
            "W1": np.asarray(W1, np.float32),
            "W2": np.asarray(W2, np.float32).astype(np.float16),
            "W3": np.asarray(W3, np.float32).astype(np.float16),
            "p12": p12, "p3": p3,
        })
    return in_maps, meta, core_of, loc_of


def kernel(x, ei, batch, W1, b1, g1, be1, W2, b2, g2, be2, W3, b3, g3, be3):
    global LAST_EXEC_NS
    from concourse.bass_utils import run_bass_kernel_spmd

    N, NP = _CFG["N"], _CFG["NP"]
    in_maps, meta, core_of, loc_of = _prepare(
        x, ei, W1, g1, be1, W2, g2, be2, W3, g3, be3)
    nc = _build_nc(meta, in_maps[0]["idx"].shape[1], in_maps[0]["dst"].shape[1])

    trace = bool(int(os.environ.get("GCN_TRACE", "0")))
    res = run_bass_kernel_spmd(nc, in_maps, list(range(NCORE)), trace=trace)
    if res.exec_time_ns is not None:
        LAST_EXEC_NS = res.exec_time_ns

    out = np.empty((N, DIN), np.float32)
    for c in range(NCORE):
        nodes_c = np.nonzero(core_of == c)[0]
        arr = res.results[c]["outT"].reshape(DIN, NP)
        out[nodes_c] = arr[:, loc_of[nodes_c]].T
    return out

